# revision 8
# baseline (speedup 1.0000x reference)
"""Causal multi-head self-attention on 8 Trainium2 NeuronCores.

Problem (hardcoded): B=4, S=2048, D=1024, H=16, DK=64, fp32 in/out.

Sharding: hybrid data-parallel x tensor-parallel.
  core c -> batch b = c // 2, head-group g = c % 2 (8 heads each).
Each core computes q/k/v projections for its 8 heads, causal attention,
and a partial output projection against its 512 columns of Wo. The
host sums the two partials per batch (the Megatron all-reduce, done at
unshard time).

Per-core kernel (all matmuls in float32r, ~1e-4 relative error):
  xT [D, S] (host-transposed) -> qT/kT [512, S] with head-dim on
  partitions, v [S, 8*65] with a ones column appended per head so the
  PV matmul also produces softmax column sums (row 64 of its PSUM out).
  Scores are computed transposed ([keys, queries]) so exp tiles feed the
  PV matmul directly and z comes out as zT [512, S], the layout the Wo
  projection needs. Causality: fully-masked tiles skipped, 4 diagonal
  0/1 masks multiply the exp tiles. Normalization: reciprocal of the
  column sums, broadcast across partitions with a K=1 ones matmul, then
  one fused DVE multiply during the z PSUM eviction.
"""

import numpy as np
from contextlib import ExitStack

import concourse.bacc as bacc
import concourse.mybir as mybir
from concourse import tile
from concourse.bass_utils import run_bass_kernel_spmd

B, S, D, H, DK = 4, 2048, 1024, 16, 64
NCORES = 8
HPC = H // 2              # heads per core
HD = HPC * DK             # 512 local head-dim
P = 128
NQ = 512                  # query tile (free dim of scoresT)
VW = DK + 1               # 65: v columns per head incl. ones column
SCALE = 1.0 / np.sqrt(np.float32(DK))

f32 = mybir.dt.float32
f32r = mybir.dt.float32r
AF = mybir.ActivationFunctionType


def build(reps: int = 1):
    """Build + compile the per-core program. reps>1 repeats the whole
    compute (for slope timing in test harnesses)."""
    nc = bacc.Bacc("TRN2", target_bir_lowering=False, debug=False)

    xT = nc.dram_tensor("xT", [D, S], f32r, kind="ExternalInput").ap()
    wqT = nc.dram_tensor("wqT", [D, HD], f32r, kind="ExternalInput").ap()
    wkT = nc.dram_tensor("wkT", [D, HD], f32r, kind="ExternalInput").ap()
    wvT = nc.dram_tensor("wvT", [D, HD], f32r, kind="ExternalInput").ap()
    woT = nc.dram_tensor("woT", [HD, D], f32r, kind="ExternalInput").ap()
    ones8 = nc.dram_tensor("ones8", [P, HPC], f32r, kind="ExternalInput").ap()
    ones_row = nc.dram_tensor("ones_row", [1, 64], f32r, kind="ExternalInput").ap()
    masks = nc.dram_tensor("masks", [4, P, NQ], f32r, kind="ExternalInput").ap()
    y = nc.dram_tensor("y", [S, D], f32, kind="ExternalOutput").ap()

    ND = D // P        # 8 d-chunks
    NSC = S // NQ      # 4 s-chunks of 512
    NJT = HD // P      # 4 head-pair tiles
    NST = S // P       # 16 s-tiles of 128

    with tile.TileContext(nc) as tc, ExitStack() as ctx:
        const = ctx.enter_context(tc.tile_pool(name="const", bufs=1))
        ones8_t = const.tile([P, HPC], f32r, tag="ones8")
        oner_t = const.tile([1, 64], f32r, tag="oner")
        nc.sync.dma_start(out=ones8_t[:], in_=ones8[:])
        nc.sync.dma_start(out=oner_t[:], in_=ones_row[:])

        # persistent activations
        act = ctx.enter_context(tc.tile_pool(name="act", bufs=1))
        qT = [act.tile([P, S], f32r, tag=f"qT{j}", name=f"qT{j}") for j in range(NJT)]
        kT = [act.tile([P, S], f32r, tag=f"kT{j}", name=f"kT{j}") for j in range(NJT)]
        v = [act.tile([P, HPC * VW], f32r, tag=f"v{t}", name=f"v{t}")
             for t in range(NST)]

        for _rep in range(reps):
            # ---------------- phase 1: q/k/v projections ----------------
            with tc.tile_pool(name="w", bufs=1) as wpool, \
                 tc.tile_pool(name="xs", bufs=2) as xpool, \
                 tc.tile_pool(name="pj", bufs=2, space="PSUM") as pj:
                wq_t = [wpool.tile([P, HD], f32r, tag=f"wq{d}", name=f"wq{d}")
                        for d in range(ND)]
                wk_t = [wpool.tile([P, HD], f32r, tag=f"wk{d}", name=f"wk{d}")
                        for d in range(ND)]
                wv_t = [wpool.tile([P, HD], f32r, tag=f"wv{d}", name=f"wv{d}")
                        for d in range(ND)]
                for d in range(ND):
                    nc.sync.dma_start(out=wq_t[d][:], in_=wqT[d * P:(d + 1) * P, :])
                    nc.sync.dma_start(out=wk_t[d][:], in_=wkT[d * P:(d + 1) * P, :])
                    nc.sync.dma_start(out=wv_t[d][:], in_=wvT[d * P:(d + 1) * P, :])

                for sc in range(NSC):
                    ssl = slice(sc * NQ, (sc + 1) * NQ)
                    xs = [xpool.tile([P, NQ], f32r, tag=f"xs{d}", name=f"xs{d}")
                          for d in range(ND)]
                    for d in range(ND):
                        nc.sync.dma_start(out=xs[d][:], in_=xT[d * P:(d + 1) * P, ssl])
                    for jt in range(NJT):
                        jsl = slice(jt * P, (jt + 1) * P)
                        pq = pj.tile([P, NQ], f32, tag="pq")
                        pk = pj.tile([P, NQ], f32, tag="pk")
                        for d in range(ND):
                            nc.tensor.matmul(pq[:], wq_t[d][:, jsl], xs[d][:],
                                             start=(d == 0), stop=(d == ND - 1))
                        for d in range(ND):
                            nc.tensor.matmul(pk[:], wk_t[d][:, jsl], xs[d][:],
                                             start=(d == 0), stop=(d == ND - 1))
                        nc.vector.tensor_copy(qT[jt][:, ssl], pq[:])
                        nc.vector.tensor_copy(kT[jt][:, ssl], pk[:])
                    for st in range(NQ // P):
                        t = sc * (NQ // P) + st
                        pv = pj.tile([P, HD], f32, tag="pv")
                        for d in range(ND):
                            nc.tensor.matmul(pv[:], xs[d][:, st * P:(st + 1) * P],
                                             wv_t[d][:],
                                             start=(d == 0), stop=(d == ND - 1))
                        # strided copy: per-head 64 v columns into 65-wide slots
                        v3 = v[t][:].rearrange("p (h x) -> p h x", x=VW)[:, :, 0:DK]
                        pv3 = pv[:].rearrange("p (h x) -> p h x", x=DK)
                        nc.vector.tensor_copy(v3, pv3)
                        # ones column per head (col 64 of each 65-slot)
                        o3 = v[t][:].rearrange("p (h x) -> p h x", x=VW)[:, :, DK:VW]
                        nc.sync.dma_start(out=o3, in_=ones8[:].unsqueeze(2))

            # ---------------- phase 2: attention per head-pair ----------------
            p23 = ExitStack()
            zpool = p23.enter_context(tc.tile_pool(name="zpool", bufs=1))
            zT = [zpool.tile([P, S], f32r, tag=f"zT{j}", name=f"zT{j}")
                  for j in range(NJT)]
            with tc.tile_pool(name="ep", bufs=6) as epool, \
                 tc.tile_pool(name="msk", bufs=1) as mpool, \
                 tc.tile_pool(name="sps", bufs=2, space="PSUM") as sps, \
                 tc.tile_pool(name="zps", bufs=1, space="PSUM") as zps, \
                 tc.tile_pool(name="bcps", bufs=1, space="PSUM") as bcps, \
                 tc.tile_pool(name="sm", bufs=4) as smpool:
                mask_t = [mpool.tile([P, NQ], f32r, tag=f"mask{r}", name=f"mask{r}")
                          for r in range(4)]
                for r in range(4):
                    nc.sync.dma_start(out=mask_t[r][:], in_=masks[r])
                for hp in range(NJT):
                    for ti in range(S // NQ):
                        isl = slice(ti * NQ, (ti + 1) * NQ)
                        jmax = (ti + 1) * (NQ // P)
                        zpA = zps.tile([P, NQ], f32, tag="zpA", name="zpA")
                        zpB = zps.tile([P, NQ], f32, tag="zpB", name="zpB")
                        for tj in range(jmax):
                            jsl = slice(tj * P, (tj + 1) * P)
                            sp = sps.tile([P, 2 * NQ], f32, tag="sp")
                            # scoresT: head A on PE rows 0-63, head B on rows
                            # 64-127 (concurrent row groups)
                            nc.tensor.matmul(sp[:, 0:NQ],
                                             kT[hp][0:64, jsl], qT[hp][0:64, isl],
                                             start=True, stop=True)
                            nc.tensor.matmul(sp[:, NQ:2 * NQ],
                                             kT[hp][64:128, jsl], qT[hp][64:128, isl],
                                             start=True, stop=True,
                                             tile_position=(64, 0))
                            ep = epool.tile([P, 2 * NQ], f32r, tag="ep")
                            nc.scalar.activation(ep[:], sp[:], AF.Exp,
                                                 scale=float(SCALE))
                            r = tj - 4 * ti
                            if r >= 0:
                                nc.vector.tensor_mul(ep[:, 0:NQ], ep[:, 0:NQ],
                                                     mask_t[r][:])
                                nc.vector.tensor_mul(ep[:, NQ:2 * NQ],
                                                     ep[:, NQ:2 * NQ], mask_t[r][:])
                            first, last = (tj == 0), (tj == jmax - 1)
                            # PV matmul, M=65: rows 0-63 = zT, row 64 = colsum
                            hA, hB = 2 * hp, 2 * hp + 1
                            nc.tensor.matmul(zpA[0:VW, :],
                                             v[tj][:, hA * VW:(hA + 1) * VW],
                                             ep[:, 0:NQ],
                                             start=first, stop=last)
                            nc.tensor.matmul(zpB[0:VW, :],
                                             v[tj][:, hB * VW:(hB + 1) * VW],
                                             ep[:, NQ:2 * NQ],
                                             start=first, stop=last)
                        recA = smpool.tile([1, NQ], f32r, tag="recA")
                        recB = smpool.tile([1, NQ], f32r, tag="recB")
                        with nc.allow_low_precision(reason="f32r is f32 storage"):
                            nc.vector.reciprocal(recA[:], zpA[DK:VW, :])
                            nc.vector.reciprocal(recB[:], zpB[DK:VW, :])
                        bcA = bcps.tile([64, NQ], f32, tag="bcA", name="bcA")
                        bcB = bcps.tile([64, NQ], f32, tag="bcB", name="bcB")
                        nc.tensor.matmul(bcA[:], oner_t[:], recA[:],
                                         start=True, stop=True)
                        nc.tensor.matmul(bcB[:], oner_t[:], recB[:],
                                         start=True, stop=True)
                        bcs = smpool.tile([P, NQ], f32, tag="bcs")
                        nc.vector.tensor_copy(bcs[0:64, :], bcA[:])
                        nc.vector.tensor_copy(bcs[64:128, :], bcB[:])
                        nc.vector.tensor_mul(zT[hp][0:64, isl], zpA[0:64, :],
                                             bcs[0:64, :])
                        nc.vector.tensor_mul(zT[hp][64:128, isl], zpB[0:64, :],
                                             bcs[64:128, :])

            # ---------------- phase 3: output projection ----------------
            with tc.tile_pool(name="wo", bufs=1) as wopool, \
                 tc.tile_pool(name="yp", bufs=3) as ypool, \
                 tc.tile_pool(name="py", bufs=4, space="PSUM") as pyp:
                wo_t = [wopool.tile([P, D], f32r, tag=f"wo{j}", name=f"wo{j}")
                        for j in range(NJT)]
                for j in range(NJT):
                    nc.sync.dma_start(out=wo_t[j][:], in_=woT[j * P:(j + 1) * P, :])
                for st in range(NST):
                    stsl = slice(st * P, (st + 1) * P)
                    for ot in range(D // NQ):
                        osl = slice(ot * NQ, (ot + 1) * NQ)
                        py = pyp.tile([P, NQ], f32, tag="py")
                        for j in range(NJT):
                            nc.tensor.matmul(py[:], zT[j][:, stsl], wo_t[j][:, osl],
                                             start=(j == 0), stop=(j == NJT - 1))
                        ys = ypool.tile([P, NQ], f32, tag="ys")
                        nc.vector.tensor_copy(ys[:], py[:])
                        nc.sync.dma_start(out=y[stsl, osl], in_=ys[:])
            p23.close()

    nc.compile()
    return nc


def _host_inputs(x, Wq, Wk, Wv, Wo):
    """Per-core input dicts (host-side shard + transpose)."""
    x = np.asarray(x, dtype=np.float32)
    Wq = np.asarray(Wq, dtype=np.float32)
    Wk = np.asarray(Wk, dtype=np.float32)
    Wv = np.asarray(Wv, dtype=np.float32)
    Wo = np.asarray(Wo, dtype=np.float32)

    ones8 = np.ones((P, HPC), np.float32)
    ones_row = np.ones((1, 64), np.float32)
    masks = np.zeros((4, P, NQ), np.float32)
    jj = np.arange(P)[:, None]
    ii = np.arange(NQ)[None, :]
    for r in range(4):
        masks[r] = ((jj + 128 * r) <= ii).astype(np.float32)

    in_maps = []
    xT_cache = {}
    for c in range(NCORES):
        b, g = c // 2, c % 2
        if b not in xT_cache:
            xT_cache[b] = np.ascontiguousarray(x[b].T)
        hs = slice(g * HPC, (g + 1) * HPC)
        wqT = np.ascontiguousarray(Wq[hs].transpose(2, 0, 1).reshape(D, HD))
        wkT = np.ascontiguousarray(Wk[hs].transpose(2, 0, 1).reshape(D, HD))
        wvT = np.ascontiguousarray(Wv[hs].transpose(2, 0, 1).reshape(D, HD))
        woT = np.ascontiguousarray(Wo[:, g * HD:(g + 1) * HD].T)
        in_maps.append({
            "xT": xT_cache[b], "wqT": wqT, "wkT": wkT, "wvT": wvT,
            "woT": woT, "ones8": ones8, "ones_row": ones_row,
            "masks": masks,
        })
    return in_maps


_NC_CACHE = {}


def kernel(x, Wq, Wk, Wv, Wo):
    if "nc" not in _NC_CACHE:
        _NC_CACHE["nc"] = build()
    nc = _NC_CACHE["nc"]
    in_maps = _host_inputs(x, Wq, Wk, Wv, Wo)
    res = run_bass_kernel_spmd(nc, in_maps, list(range(NCORES)))
    out = np.empty((B, S, D), np.float32)
    for b in range(B):
        out[b] = res.results[2 * b]["y"] + res.results[2 * b + 1]["y"]
    return out


# revision 9
# speedup vs baseline: 1039.0490x; 1039.0490x over previous
"""Causal multi-head self-attention on 8 Trainium2 NeuronCores.

Problem (hardcoded): B=4, S=2048, D=1024, H=16, DK=64, fp32 in/out.

Sharding: hybrid data-parallel x tensor-parallel.
  core c -> batch b = c // 2, head-group g = c % 2 (8 heads each).
Each core computes q/k/v projections for its 8 heads, causal attention,
and a partial output projection against its 512 columns of Wo. The
host sums the two partials per batch (the Megatron all-reduce, done at
unshard time).

Per-core kernel (all matmuls in float32r, ~1e-4 relative error):
  xT [D, S] (host-transposed) -> qT/kT [512, S] with head-dim on
  partitions, v [S, 8*65] with a ones column appended per head so the
  PV matmul also produces softmax column sums (row 64 of its PSUM out).
  Scores are computed transposed ([keys, queries]) so exp tiles feed the
  PV matmul directly and z comes out as zT [512, S], the layout the Wo
  projection needs. Causality: fully-masked tiles skipped, 4 diagonal
  0/1 masks multiply the exp tiles. Normalization: reciprocal of the
  column sums, broadcast across partitions with a K=1 ones matmul, then
  one fused DVE multiply during the z PSUM eviction.
"""

import numpy as np
from contextlib import ExitStack

import concourse.bacc as bacc
import concourse.mybir as mybir
from concourse import tile
from concourse.bass_utils import run_bass_kernel_spmd

B, S, D, H, DK = 4, 2048, 1024, 16, 64
NCORES = 8
HPC = H // 2              # heads per core
HD = HPC * DK             # 512 local head-dim
P = 128
NQ = 512                  # query tile (free dim of scoresT)
VW = DK + 1               # 65: v columns per head incl. ones column
SCALE = 1.0 / np.sqrt(np.float32(DK))

f32 = mybir.dt.float32
f32r = mybir.dt.float32r
AF = mybir.ActivationFunctionType


def build(reps: int = 1):
    """Build + compile the per-core program. reps>1 repeats the whole
    compute (for slope timing in test harnesses)."""
    nc = bacc.Bacc("TRN2", target_bir_lowering=False, debug=False)

    xT = nc.dram_tensor("xT", [D, S], f32r, kind="ExternalInput").ap()
    wqT = nc.dram_tensor("wqT", [D, HD], f32r, kind="ExternalInput").ap()
    wkT = nc.dram_tensor("wkT", [D, HD], f32r, kind="ExternalInput").ap()
    wvT = nc.dram_tensor("wvT", [D, HD], f32r, kind="ExternalInput").ap()
    woT = nc.dram_tensor("woT", [HD, D], f32r, kind="ExternalInput").ap()
    ones8 = nc.dram_tensor("ones8", [P, HPC], f32r, kind="ExternalInput").ap()
    ones_row = nc.dram_tensor("ones_row", [1, 64], f32r, kind="ExternalInput").ap()
    masks = nc.dram_tensor("masks", [4, P, NQ], f32r, kind="ExternalInput").ap()
    y = nc.dram_tensor("y", [S, D], f32, kind="ExternalOutput").ap()

    ND = D // P        # 8 d-chunks
    NSC = S // NQ      # 4 s-chunks of 512
    NJT = HD // P      # 4 head-pair tiles
    NST = S // P       # 16 s-tiles of 128

    with tile.TileContext(nc) as tc, ExitStack() as ctx:
        const = ctx.enter_context(tc.tile_pool(name="const", bufs=1))
        ones8_t = const.tile([P, HPC], f32r, tag="ones8")
        oner_t = const.tile([1, 64], f32r, tag="oner")
        nc.sync.dma_start(out=ones8_t[:], in_=ones8[:])
        nc.sync.dma_start(out=oner_t[:], in_=ones_row[:])

        # persistent activations
        act = ctx.enter_context(tc.tile_pool(name="act", bufs=1))
        qT = [act.tile([P, S], f32r, tag=f"qT{j}", name=f"qT{j}") for j in range(NJT)]
        kT = [act.tile([P, S], f32r, tag=f"kT{j}", name=f"kT{j}") for j in range(NJT)]
        v = [act.tile([P, HPC * VW], f32r, tag=f"v{t}", name=f"v{t}")
             for t in range(NST)]

        for _rep in range(reps):
            # ---------------- phase 1: q/k/v projections ----------------
            with tc.tile_pool(name="w", bufs=1) as wpool, \
                 tc.tile_pool(name="xs", bufs=2) as xpool, \
                 tc.tile_pool(name="pj", bufs=2, space="PSUM") as pj:
                wq_t = [wpool.tile([P, HD], f32r, tag=f"wq{d}", name=f"wq{d}")
                        for d in range(ND)]
                wk_t = [wpool.tile([P, HD], f32r, tag=f"wk{d}", name=f"wk{d}")
                        for d in range(ND)]
                wv_t = [wpool.tile([P, HD], f32r, tag=f"wv{d}", name=f"wv{d}")
                        for d in range(ND)]
                for d in range(ND):
                    nc.sync.dma_start(out=wq_t[d][:], in_=wqT[d * P:(d + 1) * P, :])
                    nc.sync.dma_start(out=wk_t[d][:], in_=wkT[d * P:(d + 1) * P, :])
                    nc.sync.dma_start(out=wv_t[d][:], in_=wvT[d * P:(d + 1) * P, :])

                for sc in range(NSC):
                    ssl = slice(sc * NQ, (sc + 1) * NQ)
                    xs = [xpool.tile([P, NQ], f32r, tag=f"xs{d}", name=f"xs{d}")
                          for d in range(ND)]
                    for d in range(ND):
                        nc.sync.dma_start(out=xs[d][:], in_=xT[d * P:(d + 1) * P, ssl])
                    for jt in range(NJT):
                        jsl = slice(jt * P, (jt + 1) * P)
                        pq = pj.tile([P, NQ], f32, tag="pq")
                        pk = pj.tile([P, NQ], f32, tag="pk")
                        for d in range(ND):
                            nc.tensor.matmul(pq[:], wq_t[d][:, jsl], xs[d][:],
                                             start=(d == 0), stop=(d == ND - 1))
                        for d in range(ND):
                            nc.tensor.matmul(pk[:], wk_t[d][:, jsl], xs[d][:],
                                             start=(d == 0), stop=(d == ND - 1))
                        nc.vector.tensor_copy(qT[jt][:, ssl], pq[:])
                        nc.vector.tensor_copy(kT[jt][:, ssl], pk[:])
                    for st in range(NQ // P):
                        t = sc * (NQ // P) + st
                        pv = pj.tile([P, HD], f32, tag="pv")
                        for d in range(ND):
                            nc.tensor.matmul(pv[:], xs[d][:, st * P:(st + 1) * P],
                                             wv_t[d][:],
                                             start=(d == 0), stop=(d == ND - 1))
                        # strided copy: per-head 64 v columns into 65-wide slots
                        v3 = v[t][:].rearrange("p (h x) -> p h x", x=VW)[:, :, 0:DK]
                        pv3 = pv[:].rearrange("p (h x) -> p h x", x=DK)
                        nc.vector.tensor_copy(v3, pv3)
                        # ones column per head (col 64 of each 65-slot)
                        o3 = v[t][:].rearrange("p (h x) -> p h x", x=VW)[:, :, DK:VW]
                        nc.sync.dma_start(out=o3, in_=ones8[:].unsqueeze(2))

            # ------- phases 2+3: attention (ti-outer) + fused y projection -------
            # ti-outer ordering lets the Wo projection of query block ti
            # fill PE while ScalarE exp (the bottleneck) works on block ti+1.
            p23 = ExitStack()
            zpool = p23.enter_context(tc.tile_pool(name="zpool", bufs=1))
            zT = [zpool.tile([P, S], f32r, tag=f"zT{j}", name=f"zT{j}")
                  for j in range(NJT)]
            with tc.tile_pool(name="ep", bufs=4) as epool, \
                 tc.tile_pool(name="msk", bufs=1) as mpool, \
                 tc.tile_pool(name="wo", bufs=1) as wopool, \
                 tc.tile_pool(name="yp", bufs=3) as ypool, \
                 tc.tile_pool(name="sps", bufs=2, space="PSUM") as sps, \
                 tc.tile_pool(name="zps", bufs=1, space="PSUM") as zps, \
                 tc.tile_pool(name="pyps", bufs=2, space="PSUM") as pyps, \
                 tc.tile_pool(name="sm", bufs=4) as smpool:
                mask_t = [mpool.tile([P, NQ], f32r, tag=f"mask{r}", name=f"mask{r}")
                          for r in range(4)]
                for r in range(4):
                    nc.sync.dma_start(out=mask_t[r][:], in_=masks[r])
                wo_t = [wopool.tile([P, D], f32r, tag=f"wo{j}", name=f"wo{j}")
                        for j in range(NJT)]
                for j in range(NJT):
                    nc.sync.dma_start(out=wo_t[j][:], in_=woT[j * P:(j + 1) * P, :])
                for ti in range(S // NQ):
                    isl = slice(ti * NQ, (ti + 1) * NQ)
                    jmax = (ti + 1) * (NQ // P)
                    for hp in range(NJT):
                        zpA = zps.tile([P, NQ], f32, tag="zpA", name="zpA")
                        zpB = zps.tile([P, NQ], f32, tag="zpB", name="zpB")
                        for tj in range(jmax):
                            jsl = slice(tj * P, (tj + 1) * P)
                            sp = sps.tile([P, 2 * NQ], f32, tag="sp")
                            # scoresT: head A on PE rows 0-63, head B on rows
                            # 64-127 (concurrent row groups)
                            nc.tensor.matmul(sp[:, 0:NQ],
                                             kT[hp][0:64, jsl], qT[hp][0:64, isl],
                                             start=True, stop=True)
                            nc.tensor.matmul(sp[:, NQ:2 * NQ],
                                             kT[hp][64:128, jsl], qT[hp][64:128, isl],
                                             start=True, stop=True,
                                             tile_position=(64, 0))
                            ep = epool.tile([P, 2 * NQ], f32r, tag="ep")
                            nc.scalar.activation(ep[:], sp[:], AF.Exp,
                                                 scale=float(SCALE))
                            r = tj - 4 * ti
                            if r >= 0:
                                nc.vector.tensor_mul(ep[:, 0:NQ], ep[:, 0:NQ],
                                                     mask_t[r][:])
                                nc.vector.tensor_mul(ep[:, NQ:2 * NQ],
                                                     ep[:, NQ:2 * NQ], mask_t[r][:])
                            first, last = (tj == 0), (tj == jmax - 1)
                            # PV matmul, M=65: rows 0-63 = zT, row 64 = colsum
                            hA, hB = 2 * hp, 2 * hp + 1
                            nc.tensor.matmul(zpA[0:VW, :],
                                             v[tj][:, hA * VW:(hA + 1) * VW],
                                             ep[:, 0:NQ],
                                             start=first, stop=last)
                            nc.tensor.matmul(zpB[0:VW, :],
                                             v[tj][:, hB * VW:(hB + 1) * VW],
                                             ep[:, NQ:2 * NQ],
                                             start=first, stop=last)
                        recA = smpool.tile([1, NQ], f32r, tag="recA")
                        recB = smpool.tile([1, NQ], f32r, tag="recB")
                        with nc.allow_low_precision(reason="f32r is f32 storage"):
                            nc.vector.reciprocal(recA[:], zpA[DK:VW, :])
                            nc.vector.reciprocal(recB[:], zpB[DK:VW, :])
                        bcA = pyps.tile([64, NQ], f32, tag="py", name="bcA")
                        nc.tensor.matmul(bcA[:], oner_t[:], recA[:],
                                         start=True, stop=True)
                        bcB = pyps.tile([64, NQ], f32, tag="py", name="bcB")
                        nc.tensor.matmul(bcB[:], oner_t[:], recB[:],
                                         start=True, stop=True)
                        bcs = smpool.tile([P, NQ], f32, tag="bcs")
                        nc.vector.tensor_copy(bcs[0:64, :], bcA[:])
                        nc.vector.tensor_copy(bcs[64:128, :], bcB[:])
                        nc.vector.tensor_mul(zT[hp][0:64, isl], zpA[0:64, :],
                                             bcs[0:64, :])
                        nc.vector.tensor_mul(zT[hp][64:128, isl], zpB[0:64, :],
                                             bcs[64:128, :])
                    # y projection for the s-tiles of this query block
                    for st in range(4 * ti, 4 * ti + 4):
                        stsl = slice(st * P, (st + 1) * P)
                        for ot in range(D // NQ):
                            osl = slice(ot * NQ, (ot + 1) * NQ)
                            py = pyps.tile([P, NQ], f32, tag="py", name="py")
                            for j in range(NJT):
                                nc.tensor.matmul(py[:], zT[j][:, stsl],
                                                 wo_t[j][:, osl],
                                                 start=(j == 0), stop=(j == NJT - 1))
                            ys = ypool.tile([P, NQ], f32, tag="ys")
                            nc.vector.tensor_copy(ys[:], py[:])
                            nc.sync.dma_start(out=y[stsl, osl], in_=ys[:])
            p23.close()

    nc.compile()
    return nc


def _host_inputs(x, Wq, Wk, Wv, Wo):
    """Per-core input dicts (host-side shard + transpose)."""
    x = np.asarray(x, dtype=np.float32)
    Wq = np.asarray(Wq, dtype=np.float32)
    Wk = np.asarray(Wk, dtype=np.float32)
    Wv = np.asarray(Wv, dtype=np.float32)
    Wo = np.asarray(Wo, dtype=np.float32)

    ones8 = np.ones((P, HPC), np.float32)
    ones_row = np.ones((1, 64), np.float32)
    masks = np.zeros((4, P, NQ), np.float32)
    jj = np.arange(P)[:, None]
    ii = np.arange(NQ)[None, :]
    for r in range(4):
        masks[r] = ((jj + 128 * r) <= ii).astype(np.float32)

    in_maps = []
    xT_cache = {}
    for c in range(NCORES):
        b, g = c // 2, c % 2
        if b not in xT_cache:
            xT_cache[b] = np.ascontiguousarray(x[b].T)
        hs = slice(g * HPC, (g + 1) * HPC)
        wqT = np.ascontiguousarray(Wq[hs].transpose(2, 0, 1).reshape(D, HD))
        wkT = np.ascontiguousarray(Wk[hs].transpose(2, 0, 1).reshape(D, HD))
        wvT = np.ascontiguousarray(Wv[hs].transpose(2, 0, 1).reshape(D, HD))
        woT = np.ascontiguousarray(Wo[:, g * HD:(g + 1) * HD].T)
        in_maps.append({
            "xT": xT_cache[b], "wqT": wqT, "wkT": wkT, "wvT": wvT,
            "woT": woT, "ones8": ones8, "ones_row": ones_row,
            "masks": masks,
        })
    return in_maps


_NC_CACHE = {}


def kernel(x, Wq, Wk, Wv, Wo):
    if "nc" not in _NC_CACHE:
        _NC_CACHE["nc"] = build()
    nc = _NC_CACHE["nc"]
    in_maps = _host_inputs(x, Wq, Wk, Wv, Wo)
    res = run_bass_kernel_spmd(nc, in_maps, list(range(NCORES)))
    out = np.empty((B, S, D), np.float32)
    for b in range(B):
        out[b] = res.results[2 * b]["y"] + res.results[2 * b + 1]["y"]
    return out
